# revision 7
# baseline (speedup 1.0000x reference)
"""AttentionBlock (GroupNorm + 1x1-conv self-attention + proj + residual) on 8 TRN2 cores.

Strategy: data-parallel over batch (16 samples -> 2 per core). Per sample, the
whole block runs out of SBUF:
  - GroupNorm(8 groups): per-partition bn_stats on DVE, cross-partition group
    sums via tiny ones-matmuls on PE, affine applied as xn = (x - m) * rstd.
  - q, k computed in [c, n] layout; v computed directly transposed ([m, c]) by
    swapping the matmul operand roles (lhsT = xn, rhs = w_vT) - no transposes.
  - attention logits computed transposed: logitsT[m, n] = sum_c k[c,m] q[c,n]
    (lhsT = k, rhs = q), exp on ScalarE with the 1/sqrt(c) scale folded in
    (softmax max-subtraction skipped: scaled logits are ~N(0,1), exp is safe),
    denominator via ones-matmul, normalization deferred past AV and proj
    (both are linear), fused into the final residual-add pass on VectorE.
  - All matmuls run in float32r (full PE rate, ~1e-4 rounding).
"""

import os
import sys

for _p in ("/root/.axon_site", "/root/.axon_site/_ro/trn_rl_repo", "/opt/trn_rl_repo"):
    if os.path.isdir(_p) and _p not in sys.path:
        sys.path.append(_p)

import numpy as np

import concourse.bass as bass
import concourse.tile as tile
from concourse import bacc, mybir
from concourse.bass_utils import run_bass_kernel_spmd

N_CORES = 8
B, C, H, W = 16, 512, 32, 32
HW = H * W                  # 1024 spatial positions
BPC = B // N_CORES          # samples per core
CO = C // 128               # 4 channel tiles
MT = HW // 128              # 8 spatial (m) tiles
NCH = HW // 512             # 2 free-dim chunks of 512
GROUPS = 8
EPS = 1e-5
SCALE = float(C) ** -0.5

F32 = mybir.dt.float32
F32R = mybir.dt.float32r
AF = mybir.ActivationFunctionType

TRACE = False               # test.py sets kernel.TRACE = True for NTFF timing

_CACHE: dict = {}


def _build(flags):
    has_qb, has_kb, has_vb, has_pb, has_gamma, has_beta = flags
    nc = bacc.Bacc(trn_type="TRN2", target_bir_lowering=False, debug=False,
                   num_devices=N_CORES)

    xs = nc.dram_tensor("xs", [BPC, C, HW], F32, kind="ExternalInput").ap()
    wt = nc.dram_tensor("wt", [C, 3 * C], F32R, kind="ExternalInput").ap()    # qkv_w^T
    pwt = nc.dram_tensor("pwt", [C, C], F32R, kind="ExternalInput").ap()      # proj_w^T
    gam = nc.dram_tensor("gam", [128, CO], F32, kind="ExternalInput").ap()
    bet = nc.dram_tensor("bet", [128, CO], F32, kind="ExternalInput").ap()
    qbt = nc.dram_tensor("qbt", [128, 3 * CO], F32, kind="ExternalInput").ap()
    vb = nc.dram_tensor("vb", [1, C], F32, kind="ExternalInput").ap()
    pbt = nc.dram_tensor("pbt", [128, CO], F32, kind="ExternalInput").ap()
    out = nc.dram_tensor("out", [BPC, C, HW], F32, kind="ExternalOutput").ap()

    with tile.TileContext(nc) as tc:
        with (
            tc.tile_pool(name="wpool", bufs=1) as wpool,
            tc.tile_pool(name="xfp", bufs=2) as xfp,
            tc.tile_pool(name="xno", bufs=2) as xno,      # xn / out_sb share slots
            tc.tile_pool(name="big", bufs=1) as big,
            tc.tile_pool(name="half", bufs=1) as half,    # per-n-half tensors
            tc.tile_pool(name="small", bufs=4) as small,
            tc.tile_pool(name="mmps", bufs=4, space="PSUM") as mmps,
            tc.tile_pool(name="dps", bufs=2, space="PSUM") as dps,
        ):
            # ---- persistent weights ----
            wt_sb = wpool.tile([128, CO, 3 * C], F32R, tag="wt")
            nc.sync.dma_start(wt_sb, wt.rearrange("(co p) o -> p co o", p=128))
            pwt_sb = wpool.tile([128, CO, C], F32R, tag="pwt")
            nc.sync.dma_start(pwt_sb, pwt.rearrange("(ci p) o -> p ci o", p=128))

            gam_sb = wpool.tile([128, CO], F32, tag="gam")
            nc.sync.dma_start(gam_sb, gam)
            bet_sb = wpool.tile([128, CO], F32, tag="bet")
            nc.sync.dma_start(bet_sb, bet)
            qbt_sb = wpool.tile([128, 3 * CO], F32, tag="qbt")
            nc.sync.dma_start(qbt_sb, qbt)
            pbt_sb = wpool.tile([128, CO], F32, tag="pbt")
            nc.sync.dma_start(pbt_sb, pbt)
            if has_vb:
                vb1 = wpool.tile([1, C], F32, tag="vb1")
                nc.sync.dma_start(vb1, vb)
                vb_bc = wpool.tile([128, C], F32, tag="vbbc")
                nc.gpsimd.partition_broadcast(vb_bc, vb1)

            onesf = wpool.tile([128, 128], F32, tag="onesf")
            nc.vector.memset(onesf, 1.0)
            ones_r = wpool.tile([128, 128], F32R, tag="onesr")
            nc.vector.tensor_copy(ones_r, onesf)
            eps_sb = wpool.tile([1, 1], F32, tag="eps")
            nc.vector.memset(eps_sb, EPS)
            # per-partition group-half masks: lo = partitions 0-63, hi = 64-127
            mask_lo = wpool.tile([128, 1], F32, tag="mlo")
            nc.vector.memset(mask_lo, 0.0)
            nc.vector.memset(mask_lo[0:64], 1.0)
            mask_hi = wpool.tile([128, 1], F32, tag="mhi")
            nc.vector.tensor_sub(mask_hi, onesf[:, 0:1], mask_lo)

            for s in range(BPC):
                # ================= load x =================
                xf = xfp.tile([128, CO, HW], F32, tag="xf")
                nc.sync.dma_start(xf, xs[s].rearrange("(co p) n -> p co n", p=128))

                # ================= group norm =================
                # per-partition stats over the 1024 spatial elems
                # sp_wide[:, co, :] = [m*lo, E2*lo, m*hi, E2*hi] so a single
                # full-K ones-matmul yields every group sum on psum row 0
                # (M=1 / partial-K matmuls abort on this hw, so mask instead).
                sp_wide = small.tile([128, CO * 4], F32R, tag="spw")
                for co in range(CO):
                    st = small.tile([128, 2, 6], F32, tag="bnst")
                    for ch in range(2):
                        nc.vector.bn_stats(st[:, ch], xf[:, co, ch * 512:(ch + 1) * 512])
                    mv = small.tile([128, 2], F32, tag="mv")
                    nc.vector.bn_aggr(mv, st)
                    # sp = [mean, var + mean^2] (= [mean, E[x^2]])
                    sp = small.tile([128, 2], F32, tag="sp")
                    sq = small.tile([128, 1], F32, tag="sq")
                    nc.vector.tensor_mul(sq, mv[:, 0:1], mv[:, 0:1])
                    nc.vector.tensor_copy(sp[:, 0:1], mv[:, 0:1])
                    nc.vector.tensor_add(sp[:, 1:2], mv[:, 1:2], sq)
                    nc.vector.tensor_scalar_mul(sp_wide[:, 4 * co:4 * co + 2], sp, mask_lo)
                    nc.vector.tensor_scalar_mul(sp_wide[:, 4 * co + 2:4 * co + 4], sp, mask_hi)
                gst = dps.tile([128, 16], F32, tag="gst")
                nc.tensor.matmul(gst, ones_r, sp_wide, start=True, stop=True)
                gs = small.tile([1, 16], F32, tag="gs")
                nc.scalar.copy(gs, gst[0:1])
                # group mean M = sum/64 ; var = sum(E2)/64 - M^2 ; rstd = rsqrt(var+eps)
                gm = small.tile([1, 8], F32, tag="gm")
                nc.vector.tensor_scalar_mul(gm, gs[:, 0:16:2], 1.0 / 64.0)
                ge2 = small.tile([1, 8], F32, tag="ge2")
                nc.vector.tensor_scalar_mul(ge2, gs[:, 1:16:2], 1.0 / 64.0)
                gm2 = small.tile([1, 8], F32, tag="gm2")
                nc.vector.tensor_mul(gm2, gm, gm)
                gvar = small.tile([1, 8], F32, tag="gvar")
                nc.vector.tensor_sub(gvar, ge2, gm2)
                nc.scalar.activation(out=gvar, in_=gvar, func=AF.Sqrt,
                                     bias=eps_sb, scale=1.0)
                grstd = small.tile([1, 8], F32, tag="grstd")
                nc.vector.reciprocal(grstd, gvar)
                # broadcast to [128, CO]: partitions 0-63 get even groups, 64-127
                # odd. partition_broadcast corrupts base-64 out slices on hw, so
                # broadcast both to all 128 partitions and blend with the masks.
                aA = small.tile([128, CO], F32, tag="aA")
                bM = small.tile([128, CO], F32, tag="bM")
                for dst, src in ((aA, grstd), (bM, gm)):
                    ev = small.tile([128, CO], F32, tag="bcev")
                    od = small.tile([128, CO], F32, tag="bcod")
                    nc.gpsimd.partition_broadcast(ev, src[:, 0:8:2])
                    nc.gpsimd.partition_broadcast(od, src[:, 1:8:2])
                    nc.vector.tensor_sub(od, od, ev)
                    nc.vector.scalar_tensor_tensor(
                        out=dst, in0=od, scalar=mask_hi, in1=ev,
                        op0=mybir.AluOpType.mult, op1=mybir.AluOpType.add)
                if has_gamma:
                    nc.vector.tensor_mul(aA, aA, gam_sb)
                bB = small.tile([128, CO], F32, tag="bB")
                nc.vector.tensor_mul(bB, bM, aA)        # M * A
                if has_beta:
                    nc.vector.tensor_sub(bB, bB, bet_sb)  # M*A - beta
                # xn = x*A - (M*A - beta) = (x - M)*A + beta
                xn = xno.tile([128, CO, HW], F32R, tag="xn")
                for co in range(CO):
                    nc.vector.tensor_scalar(
                        out=xn[:, co], in0=xf[:, co],
                        scalar1=aA[:, co:co + 1], scalar2=bB[:, co:co + 1],
                        op0=mybir.AluOpType.mult, op1=mybir.AluOpType.subtract)

                # ================= q, k, vT =================
                q_sb = big.tile([128, CO, HW], F32R, tag="q")
                k_sb = big.tile([128, CO, HW], F32R, tag="k")
                for idx, (dst, base, hasb) in enumerate(
                        [(q_sb, 0, has_qb), (k_sb, C, has_kb)]):
                    for co in range(CO):
                        for nch in range(NCH):
                            ps = mmps.tile([128, 512], F32, tag="mm")
                            for ki in range(CO):
                                nc.tensor.matmul(
                                    ps,
                                    wt_sb[:, ki, base + 128 * co: base + 128 * (co + 1)],
                                    xn[:, ki, 512 * nch: 512 * (nch + 1)],
                                    start=(ki == 0), stop=(ki == CO - 1))
                            dstv = dst[:, co, 512 * nch: 512 * (nch + 1)]
                            if hasb:
                                nc.scalar.activation(
                                    out=dstv, in_=ps, func=AF.Copy,
                                    bias=qbt_sb[:, idx * CO + co: idx * CO + co + 1],
                                    scale=1.0)
                            else:
                                nc.scalar.copy(dstv, ps)
                vT = big.tile([128, MT, C], F32R, tag="vT")
                for mt in range(MT):
                    ps = mmps.tile([128, 512], F32, tag="mm")
                    for ki in range(CO):
                        nc.tensor.matmul(ps,
                                         xn[:, ki, 128 * mt: 128 * (mt + 1)],
                                         wt_sb[:, ki, 2 * C: 3 * C],
                                         start=(ki == 0), stop=(ki == CO - 1))
                    if has_vb:
                        nc.vector.tensor_add(vT[:, mt], ps, vb_bc)
                    else:
                        nc.scalar.copy(vT[:, mt], ps)

                # ============ attention, by n-half ============
                out_sb = xno.tile([128, CO, HW], F32, tag="xn")  # reuses xn slots
                for h in range(NCH):
                    hs = slice(512 * h, 512 * (h + 1))
                    e_sb = half.tile([128, MT, 512], F32R, tag="e")
                    dsum = dps.tile([128, 512], F32, tag="dsum")
                    for mt in range(MT):
                        psl = mmps.tile([128, 512], F32, tag="mm")
                        for ki in range(CO):
                            nc.tensor.matmul(psl,
                                             k_sb[:, ki, 128 * mt: 128 * (mt + 1)],
                                             q_sb[:, ki, hs],
                                             start=(ki == 0), stop=(ki == CO - 1))
                        # e = exp(scale * logitsT)  (no max subtraction needed)
                        nc.scalar.activation(out=e_sb[:, mt], in_=psl,
                                             func=AF.Exp, scale=SCALE)
                        # ones-matmul: every psum row accumulates the softmax
                        # denominator, so no cross-partition broadcast needed
                        nc.tensor.matmul(dsum, ones_r, e_sb[:, mt],
                                         start=(mt == 0), stop=(mt == MT - 1))
                    rb = half.tile([128, 512], F32, tag="rb")
                    nc.vector.reciprocal(rb, dsum)
                    # U = v @ eT (unnormalized), then proj, then fused finalize
                    u_sb = half.tile([128, CO, 512], F32R, tag="u")
                    for co in range(CO):
                        ps = mmps.tile([128, 512], F32, tag="mm")
                        for mi in range(MT):
                            nc.tensor.matmul(ps,
                                             vT[:, mi, 128 * co: 128 * (co + 1)],
                                             e_sb[:, mi],
                                             start=(mi == 0), stop=(mi == MT - 1))
                        nc.scalar.copy(u_sb[:, co], ps)
                    for oo in range(CO):
                        ps = mmps.tile([128, 512], F32, tag="mm")
                        for ci in range(CO):
                            nc.tensor.matmul(ps,
                                             pwt_sb[:, ci, 128 * oo: 128 * (oo + 1)],
                                             u_sb[:, ci],
                                             start=(ci == 0), stop=(ci == CO - 1))
                        t = small.tile([128, 512], F32, tag="fin")
                        nc.vector.tensor_mul(t, ps, rb)
                        if has_pb:
                            nc.vector.scalar_tensor_tensor(
                                out=out_sb[:, oo, hs], in0=t,
                                scalar=pbt_sb[:, oo:oo + 1], in1=xf[:, oo, hs],
                                op0=mybir.AluOpType.add, op1=mybir.AluOpType.add)
                        else:
                            nc.vector.tensor_add(out_sb[:, oo, hs], t, xf[:, oo, hs])

                nc.sync.dma_start(out[s].rearrange("(co p) n -> p co n", p=128), out_sb)

    nc.compile()
    return nc


def kernel(x, norm_w, norm_b, qkv_w, qkv_b, proj_w, proj_b):
    x = np.ascontiguousarray(np.asarray(x, dtype=np.float32).reshape(B, C, HW))
    norm_w = np.asarray(norm_w, dtype=np.float32)
    norm_b = np.asarray(norm_b, dtype=np.float32)
    qkv_b = np.asarray(qkv_b, dtype=np.float32)
    proj_b = np.asarray(proj_b, dtype=np.float32)

    flags = (
        bool(qkv_b[0:C].any()), bool(qkv_b[C:2 * C].any()), bool(qkv_b[2 * C:].any()),
        bool(proj_b.any()), bool((norm_w != 1.0).any()), bool(norm_b.any()),
    )
    if flags not in _CACHE:
        _CACHE[flags] = _build(flags)
    nc = _CACHE[flags]

    wt_np = np.ascontiguousarray(np.asarray(qkv_w, dtype=np.float32).T)     # [C, 3C]
    pwt_np = np.ascontiguousarray(np.asarray(proj_w, dtype=np.float32).T)   # [C, C]
    gam_np = np.ascontiguousarray(norm_w.reshape(CO, 128).T)
    bet_np = np.ascontiguousarray(norm_b.reshape(CO, 128).T)
    qbt_np = np.ascontiguousarray(qkv_b.reshape(3 * CO, 128).T)
    vb_np = np.ascontiguousarray(qkv_b[2 * C:].reshape(1, C))
    pbt_np = np.ascontiguousarray(proj_b.reshape(CO, 128).T)

    in_maps = []
    for c in range(N_CORES):
        in_maps.append({
            "xs": x[c * BPC:(c + 1) * BPC],
            "wt": wt_np, "pwt": pwt_np,
            "gam": gam_np, "bet": bet_np, "qbt": qbt_np, "vb": vb_np, "pbt": pbt_np,
        })

    res = run_bass_kernel_spmd(nc, in_maps, core_ids=list(range(N_CORES)),
                               trace=TRACE)
    if TRACE:
        kernel.last_exec_time_ns = res.exec_time_ns
        kernel.last_mean_exec_time_ns = res.mean_exec_time_ns
        kernel.last_trace = res.instructions_and_trace
    out = np.concatenate([res.results[c]["out"] for c in range(N_CORES)], axis=0)
    return np.ascontiguousarray(out.reshape(B, C, H, W).astype(np.float32))


# revision 9
# speedup vs baseline: 1.2159x; 1.2159x over previous
"""AttentionBlock (GroupNorm + 1x1-conv self-attention + proj + residual) on 8 TRN2 cores.

Strategy: data-parallel over batch (16 samples -> 2 per core). Per sample, the
whole block runs out of SBUF:
  - GroupNorm(8 groups): per-partition bn_stats on DVE, cross-partition group
    sums via one masked full-K ones-matmul (fp32r), affine applied on DVE.
  - q, k computed in [c, n] layout; v computed directly transposed ([m, c]) by
    swapping the matmul operand roles (lhsT = xn, rhs = w_vT) - no transposes.
  - attention logits computed transposed: logitsT[m, n] = sum_c k[c,m] q[c,n]
    (lhsT = k, rhs = q), exp on ScalarE with the 1/sqrt(c) scale folded in
    (softmax max-subtraction skipped: scaled logits are ~N(0,1), exp is safe),
    denominator via ones-matmul (every psum row = denom, so no cross-partition
    broadcast), normalization deferred past AV and proj (both linear), fused
    into the final residual-add pass on VectorE.
  - Data-path matmuls run in fp16 (2 cols/cycle on the PE; chain error ~1e-4,
    measured); the tiny stats matmul stays fp32r.
"""

import os
import sys

for _p in ("/root/.axon_site", "/root/.axon_site/_ro/trn_rl_repo", "/opt/trn_rl_repo"):
    if os.path.isdir(_p) and _p not in sys.path:
        sys.path.append(_p)

import numpy as np

import concourse.bass as bass
import concourse.tile as tile
from concourse import bacc, mybir
from concourse.bass_utils import run_bass_kernel_spmd

N_CORES = 8
B, C, H, W = 16, 512, 32, 32
HW = H * W                  # 1024 spatial positions
BPC = B // N_CORES          # samples per core
CO = C // 128               # 4 channel tiles
MT = HW // 128              # 8 spatial (m) tiles
NCH = HW // 512             # 2 free-dim chunks of 512
GROUPS = 8
EPS = 1e-5
SCALE = float(C) ** -0.5

F32 = mybir.dt.float32
F32R = mybir.dt.float32r
F16 = mybir.dt.float16
AF = mybir.ActivationFunctionType

TRACE = False               # test.py sets kernel.TRACE = True for NTFF timing

_CACHE: dict = {}


def _build(flags):
    has_qb, has_kb, has_vb, has_pb, has_gamma, has_beta = flags
    nc = bacc.Bacc(trn_type="TRN2", target_bir_lowering=False, debug=False,
                   num_devices=N_CORES)

    xs = nc.dram_tensor("xs", [BPC, C, HW], F32, kind="ExternalInput").ap()
    wt = nc.dram_tensor("wt", [C, 3 * C], F16, kind="ExternalInput").ap()     # qkv_w^T
    pwt = nc.dram_tensor("pwt", [C, C], F16, kind="ExternalInput").ap()       # proj_w^T
    gam = nc.dram_tensor("gam", [128, CO], F32, kind="ExternalInput").ap()
    bet = nc.dram_tensor("bet", [128, CO], F32, kind="ExternalInput").ap()
    qbt = nc.dram_tensor("qbt", [128, 3 * CO], F32, kind="ExternalInput").ap()
    vb = nc.dram_tensor("vb", [1, C], F32, kind="ExternalInput").ap()
    pbt = nc.dram_tensor("pbt", [128, CO], F32, kind="ExternalInput").ap()
    out = nc.dram_tensor("out", [BPC, C, HW], F32, kind="ExternalOutput").ap()

    with tile.TileContext(nc) as tc:
        with (
            tc.tile_pool(name="wpool", bufs=1) as wpool,
            tc.tile_pool(name="xfp", bufs=2) as xfp,
            tc.tile_pool(name="xnp", bufs=2) as xnp,
            tc.tile_pool(name="big", bufs=1) as big,
            tc.tile_pool(name="fin", bufs=4) as finp,
            tc.tile_pool(name="small", bufs=4) as small,
            tc.tile_pool(name="mmps", bufs=6, space="PSUM") as mmps,
            tc.tile_pool(name="dps", bufs=1, space="PSUM") as dps,
        ):
            # ---- first sample's x goes first so groupnorm starts ASAP;
            #      weight DMAs overlap with it ----
            xf0 = xfp.tile([128, CO, HW], F32, tag="xf")
            xs_t = [xs[s].rearrange("(co p) n -> p co n", p=128) for s in range(BPC)]
            for co in range(CO):
                nc.sync.dma_start(xf0[:, co], xs_t[0][:, co])

            wt_sb = wpool.tile([128, CO, 3 * C], F16, tag="wt")
            nc.sync.dma_start(wt_sb, wt.rearrange("(co p) o -> p co o", p=128))
            pwt_sb = wpool.tile([128, CO, C], F16, tag="pwt")
            nc.sync.dma_start(pwt_sb, pwt.rearrange("(ci p) o -> p ci o", p=128))

            gam_sb = wpool.tile([128, CO], F32, tag="gam")
            nc.sync.dma_start(gam_sb, gam)
            bet_sb = wpool.tile([128, CO], F32, tag="bet")
            nc.sync.dma_start(bet_sb, bet)
            qbt_sb = wpool.tile([128, 3 * CO], F32, tag="qbt")
            nc.sync.dma_start(qbt_sb, qbt)
            pbt_sb = wpool.tile([128, CO], F32, tag="pbt")
            nc.sync.dma_start(pbt_sb, pbt)
            if has_vb:
                vb1 = wpool.tile([1, C], F32, tag="vb1")
                nc.sync.dma_start(vb1, vb)
                vb_bc = wpool.tile([128, C], F32, tag="vbbc")
                nc.gpsimd.partition_broadcast(vb_bc, vb1)

            onesf = wpool.tile([128, 128], F32, tag="onesf")
            nc.vector.memset(onesf, 1.0)
            ones_r = wpool.tile([128, 128], F32R, tag="onesr")
            nc.vector.tensor_copy(ones_r, onesf)
            ones16 = wpool.tile([128, 128], F16, tag="ones16")
            nc.vector.tensor_copy(ones16, onesf)
            eps_sb = wpool.tile([1, 1], F32, tag="eps")
            nc.vector.memset(eps_sb, EPS)
            # per-partition group-half masks: lo = partitions 0-63, hi = 64-127
            mask_lo = wpool.tile([128, 1], F32, tag="mlo")
            nc.vector.memset(mask_lo, 0.0)
            nc.vector.memset(mask_lo[0:64], 1.0)
            mask_hi = wpool.tile([128, 1], F32, tag="mhi")
            nc.vector.tensor_sub(mask_hi, onesf[:, 0:1], mask_lo)

            for s in range(BPC):
                # ================= load x =================
                if s == 0:
                    xf = xf0
                else:
                    xf = xfp.tile([128, CO, HW], F32, tag="xf")
                    for co in range(CO):
                        nc.sync.dma_start(xf[:, co], xs_t[s][:, co])

                # ================= group norm =================
                # sp_wide[:, 4co:4co+4] = [m*lo, E2*lo, m*hi, E2*hi] so a single
                # full-K ones-matmul yields every group sum on psum row 0
                # (M=1 / partial-K matmuls abort on this hw, so mask instead).
                sp_wide = small.tile([128, CO * 4], F32R, tag="spw")
                for co in range(CO):
                    st = small.tile([128, 2, 6], F32, tag="bnst")
                    for ch in range(2):
                        nc.vector.bn_stats(st[:, ch], xf[:, co, ch * 512:(ch + 1) * 512])
                    mv = small.tile([128, 2], F32, tag="mv")
                    nc.vector.bn_aggr(mv, st)
                    # sp = [mean, var + mean^2] (= [mean, E[x^2]])
                    sp = small.tile([128, 2], F32, tag="sp")
                    sq = small.tile([128, 1], F32, tag="sq")
                    nc.vector.tensor_mul(sq, mv[:, 0:1], mv[:, 0:1])
                    nc.vector.tensor_copy(sp[:, 0:1], mv[:, 0:1])
                    nc.vector.tensor_add(sp[:, 1:2], mv[:, 1:2], sq)
                    nc.vector.tensor_scalar_mul(sp_wide[:, 4 * co:4 * co + 2], sp, mask_lo)
                    nc.vector.tensor_scalar_mul(sp_wide[:, 4 * co + 2:4 * co + 4], sp, mask_hi)
                gst = dps.tile([128, 16], F32, tag="gst")
                nc.tensor.matmul(gst, ones_r, sp_wide, start=True, stop=True)
                gs = small.tile([1, 16], F32, tag="gs")
                nc.scalar.copy(gs, gst[0:1])
                # group mean M = sum/64 ; var = sum(E2)/64 - M^2 ; rstd = rsqrt(var+eps)
                gm = small.tile([1, 8], F32, tag="gm")
                nc.vector.tensor_scalar_mul(gm, gs[:, 0:16:2], 1.0 / 64.0)
                ge2 = small.tile([1, 8], F32, tag="ge2")
                nc.vector.tensor_scalar_mul(ge2, gs[:, 1:16:2], 1.0 / 64.0)
                gm2 = small.tile([1, 8], F32, tag="gm2")
                nc.vector.tensor_mul(gm2, gm, gm)
                gvar = small.tile([1, 8], F32, tag="gvar")
                nc.vector.tensor_sub(gvar, ge2, gm2)
                nc.scalar.activation(out=gvar, in_=gvar, func=AF.Sqrt,
                                     bias=eps_sb, scale=1.0)
                grstd = small.tile([1, 8], F32, tag="grstd")
                nc.vector.reciprocal(grstd, gvar)
                # broadcast to [128, CO]: partitions 0-63 get even groups, 64-127
                # odd. partition_broadcast corrupts base-64 out slices on hw, so
                # broadcast both to all 128 partitions and blend with the masks.
                aA = small.tile([128, CO], F32, tag="aA")
                bM = small.tile([128, CO], F32, tag="bM")
                for dst, src in ((aA, grstd), (bM, gm)):
                    ev = small.tile([128, CO], F32, tag="bcev")
                    od = small.tile([128, CO], F32, tag="bcod")
                    nc.gpsimd.partition_broadcast(ev, src[:, 0:8:2])
                    nc.gpsimd.partition_broadcast(od, src[:, 1:8:2])
                    nc.vector.tensor_sub(od, od, ev)
                    nc.vector.scalar_tensor_tensor(
                        out=dst, in0=od, scalar=mask_hi, in1=ev,
                        op0=mybir.AluOpType.mult, op1=mybir.AluOpType.add)
                if has_gamma:
                    nc.vector.tensor_mul(aA, aA, gam_sb)
                bB = small.tile([128, CO], F32, tag="bB")
                nc.vector.tensor_mul(bB, bM, aA)        # M * A
                if has_beta:
                    nc.vector.tensor_sub(bB, bB, bet_sb)  # M*A - beta
                # xn = x*A - (M*A - beta) = (x - M)*A + beta
                xn = xnp.tile([128, CO, HW], F16, tag="xn")
                for co in range(CO):
                    nc.vector.tensor_scalar(
                        out=xn[:, co], in0=xf[:, co],
                        scalar1=aA[:, co:co + 1], scalar2=bB[:, co:co + 1],
                        op0=mybir.AluOpType.mult, op1=mybir.AluOpType.subtract)

                # ================= q, k, vT =================
                q_sb = big.tile([128, CO, HW], F16, tag="q")
                k_sb = big.tile([128, CO, HW], F16, tag="k")
                for idx, (dst, base, hasb) in enumerate(
                        [(q_sb, 0, has_qb), (k_sb, C, has_kb)]):
                    for co in range(CO):
                        for nch in range(NCH):
                            ps = mmps.tile([128, 512], F32, tag="mm")
                            for ki in range(CO):
                                nc.tensor.matmul(
                                    ps,
                                    wt_sb[:, ki, base + 128 * co: base + 128 * (co + 1)],
                                    xn[:, ki, 512 * nch: 512 * (nch + 1)],
                                    start=(ki == 0), stop=(ki == CO - 1))
                            dstv = dst[:, co, 512 * nch: 512 * (nch + 1)]
                            if hasb:
                                nc.scalar.activation(
                                    out=dstv, in_=ps, func=AF.Copy,
                                    bias=qbt_sb[:, idx * CO + co: idx * CO + co + 1],
                                    scale=1.0)
                            else:
                                nc.scalar.copy(dstv, ps)
                vT = big.tile([128, MT, C], F16, tag="vT")
                for mt in range(MT):
                    ps = mmps.tile([128, 512], F32, tag="mm")
                    for ki in range(CO):
                        nc.tensor.matmul(ps,
                                         xn[:, ki, 128 * mt: 128 * (mt + 1)],
                                         wt_sb[:, ki, 2 * C: 3 * C],
                                         start=(ki == 0), stop=(ki == CO - 1))
                    if has_vb:
                        nc.vector.tensor_add(vT[:, mt], ps, vb_bc)
                    else:
                        nc.scalar.copy(vT[:, mt], ps)

                # ============ attention, by n-half ============
                e_sb = big.tile([128, MT, HW], F16, tag="e")
                u_sb = big.tile([128, CO, HW], F16, tag="u")
                rb = big.tile([128, HW], F32, tag="rb")
                out_t = out[s].rearrange("(co p) n -> p co n", p=128)
                for h in range(NCH):
                    hs = slice(512 * h, 512 * (h + 1))
                    dsum = dps.tile([128, 512], F32, tag="dsum")
                    for mt in range(MT):
                        psl = mmps.tile([128, 512], F32, tag="mm")
                        for ki in range(CO):
                            nc.tensor.matmul(psl,
                                             k_sb[:, ki, 128 * mt: 128 * (mt + 1)],
                                             q_sb[:, ki, hs],
                                             start=(ki == 0), stop=(ki == CO - 1))
                        # e = exp(scale * logitsT)  (no max subtraction needed)
                        nc.scalar.activation(out=e_sb[:, mt, hs], in_=psl,
                                             func=AF.Exp, scale=SCALE)
                        # ones-matmul: every psum row accumulates the softmax
                        # denominator, so no cross-partition broadcast needed
                        nc.tensor.matmul(dsum, ones16, e_sb[:, mt, hs],
                                         start=(mt == 0), stop=(mt == MT - 1))
                    nc.vector.reciprocal(rb[:, hs], dsum)
                    for co in range(CO):
                        ps = mmps.tile([128, 512], F32, tag="mm")
                        for mi in range(MT):
                            nc.tensor.matmul(ps,
                                             vT[:, mi, 128 * co: 128 * (co + 1)],
                                             e_sb[:, mi, hs],
                                             start=(mi == 0), stop=(mi == MT - 1))
                        nc.scalar.copy(u_sb[:, co, hs], ps)
                    for oo in range(CO):
                        ps = mmps.tile([128, 512], F32, tag="mm")
                        for ci in range(CO):
                            nc.tensor.matmul(ps,
                                             pwt_sb[:, ci, 128 * oo: 128 * (oo + 1)],
                                             u_sb[:, ci, hs],
                                             start=(ci == 0), stop=(ci == CO - 1))
                        t = finp.tile([128, 512], F32, tag="fin")
                        nc.vector.tensor_mul(t, ps, rb[:, hs])
                        fo = finp.tile([128, 512], F32, tag="fo")
                        if has_pb:
                            nc.vector.scalar_tensor_tensor(
                                out=fo, in0=t,
                                scalar=pbt_sb[:, oo:oo + 1], in1=xf[:, oo, hs],
                                op0=mybir.AluOpType.add, op1=mybir.AluOpType.add)
                        else:
                            nc.vector.tensor_add(fo, t, xf[:, oo, hs])
                        nc.sync.dma_start(out_t[:, oo, hs], fo)

    nc.compile()
    return nc


def kernel(x, norm_w, norm_b, qkv_w, qkv_b, proj_w, proj_b):
    x = np.ascontiguousarray(np.asarray(x, dtype=np.float32).reshape(B, C, HW))
    norm_w = np.asarray(norm_w, dtype=np.float32)
    norm_b = np.asarray(norm_b, dtype=np.float32)
    qkv_b = np.asarray(qkv_b, dtype=np.float32)
    proj_b = np.asarray(proj_b, dtype=np.float32)

    flags = (
        bool(qkv_b[0:C].any()), bool(qkv_b[C:2 * C].any()), bool(qkv_b[2 * C:].any()),
        bool(proj_b.any()), bool((norm_w != 1.0).any()), bool(norm_b.any()),
    )
    if flags not in _CACHE:
        _CACHE[flags] = _build(flags)
    nc = _CACHE[flags]

    wt_np = np.ascontiguousarray(np.asarray(qkv_w, dtype=np.float32).T.astype(np.float16))
    pwt_np = np.ascontiguousarray(np.asarray(proj_w, dtype=np.float32).T.astype(np.float16))
    gam_np = np.ascontiguousarray(norm_w.reshape(CO, 128).T)
    bet_np = np.ascontiguousarray(norm_b.reshape(CO, 128).T)
    qbt_np = np.ascontiguousarray(qkv_b.reshape(3 * CO, 128).T)
    vb_np = np.ascontiguousarray(qkv_b[2 * C:].reshape(1, C))
    pbt_np = np.ascontiguousarray(proj_b.reshape(CO, 128).T)

    in_maps = []
    for c in range(N_CORES):
        in_maps.append({
            "xs": x[c * BPC:(c + 1) * BPC],
            "wt": wt_np, "pwt": pwt_np,
            "gam": gam_np, "bet": bet_np, "qbt": qbt_np, "vb": vb_np, "pbt": pbt_np,
        })

    res = run_bass_kernel_spmd(nc, in_maps, core_ids=list(range(N_CORES)),
                               trace=TRACE)
    if TRACE:
        kernel.last_exec_time_ns = res.exec_time_ns
        kernel.last_mean_exec_time_ns = res.mean_exec_time_ns
        kernel.last_trace = res.instructions_and_trace
    out = np.concatenate([res.results[c]["out"] for c in range(N_CORES)], axis=0)
    return np.ascontiguousarray(out.reshape(B, C, H, W).astype(np.float32))
